# revision 72
# baseline (speedup 1.0000x reference)
"""Trainium2 Bass kernel for sliding-window attention block — v3.

Reference computation:
  x:(4,8192,1024) -> rmsnorm -> @w_qkv -> split q,k,v (16 heads, d=64)
  -> rope(q,k) -> causal local window attention (w=64, exact window)
  -> merge heads -> @w_o -> out:(4,8192,1024)

Sharding: sequence-parallel over 8 cores (1024 tokens each) with a
64-token halo of x for K/V; no collectives.

Cost-model estimate ~471.0us/core (v2 was ~542us); measured end-to-end
rel err 0.0122 on hardware (tolerance 2e-2). Rope emits k before q per
head-pair (k feeds both sim operands earlier; -0.5us).

Design (deltas from v2 marked NEW):
  * q/k projections: NEW 1-term fp8e4 DoubleRow (x8h@w8h only): the
    activation-quantization error is smoothed by softmax; halves q/k
    projection PE time. v stays 3-term (x8h@w8h + x8l@w8h + x8h@w8l)
    since v errors pass linearly to the output.
  * NEW out-projection fp8 DoubleRow 3-term: normalize emits 8*ao (Z
    ones-vector = 8), split bf16->fp8 hi/lo (Act copy + Pool subtract),
    wo pre-scaled x64 and split hi/lo host-side; 8*64 descale folded
    into the final Act Copy eviction. 25% less PE than bf16, rel-err
    neutral. ah-terms first so the Pool lo-subtract is off the phase-D
    critical path.
  * NEW window mask folded into the sim matmuls as an additive penalty:
    sim[j,i] += -112*((i-j)+ + (j-i-64)+ [+ pad]) via a K=128 bf16
    matmul of constant 0/1 x {0,-C,-2C} staircase factors -> exp gives
    ~8e-7 on masked entries; removes all 128 post-exp mask multiplies
    (~92us of DVE/Pool) and one hop from the softmax chain. NOTE: the
    fp8-DoubleRow version of this matmul passes small probes but faults
    real HW at scale when mixed into bf16 tile_position groups - keep
    it bf16.
  * NEW rmsnorm without the Act engine: per 3-tile group, E[x^2] via
    bn_stats/bn_aggr, then ONE batched rsqrt = bit-hack seed
    (0x5f3759df) + 2 Newton steps on DVE (u32 shift/sub + fused
    tensor_scalar). Act therefore only ever needs {exp, copy} -> a
    single LoadActFuncSet at startup instead of per-batch Sqrt<->Exp
    act-table thrash (was 15 loads, 19us serialized).
  * NEW all big constant DMAs ride the sync queue interleaved after
    batch-0's x-tile DMAs (the cost model serializes all DMA on one
    engine; weights used to delay phase A ~10us). v-proj terms are
    j-major so phase B starts after the first weight chunks land.
  * NEW v2 shifted copies on the gpsimd DMA queue (sync queue holds x
    tiles); rings retuned (xnbp 4, osbp 4, pT 13, rzp 5, qc 6; x8h and
    x8l single-buffered — x8h ring 2 cost only 0.6us and its 8.7KB funds
    the deeper rings).
  * Weights pre-scaled by 64 so fp8-lo residuals stay in e4m3's normal
    range; descale folded into rope tables (q,k) and Z ones (v).
  * x shipped bf16; rmsnorm + bf16 transpose on device; transposed
    tiles split to fp8 hi/lo during PSUM eviction.
  * Projection + rope + attention interleaved per head-pair; per
    (head-pair, 512-col block) the two heads share packed [128,512]
    Z/PV psum tiles -> one reciprocal + one normalize per block.
  * Elementwise balanced across DVE/Act/Pool by measured cost; Pool
    (gpsimd) only touches SBUF (hw forbids gpsimd PSUM access).
  * PSUM rings: projections 3, transposes 1, rope-perm 2, sim 1, Z/PV
    shared-tag 1 (scanned; 8 banks are fully allocated).
"""
import sys

sys.path.insert(0, "/opt/trn_rl_repo")

from contextlib import ExitStack

import numpy as np
import ml_dtypes

import concourse.bass as bass
import concourse.bacc as bacc
import concourse.tile as tile
from concourse import mybir
from concourse.bass_utils import run_bass_kernel_spmd

BF16 = ml_dtypes.bfloat16
E4 = ml_dtypes.float8_e4m3

B, N, DIM = 4, 8192, 1024
HEADS, D, W = 16, 64, 64
NCORES = 8
TS = N // NCORES          # 1024 query tokens per core
TK = TS + W               # 1088 tokens incl. halo
NWIN = TS // W            # 16 window-chunks per core
EPS = float(np.finfo(np.float32).eps)
WSCALE = 64.0             # fp8 weight pre-scale (descaled downstream)

F32 = mybir.dt.float32
BF = mybir.dt.bfloat16
FP8 = mybir.dt.float8e4


def _build_tables():
    """Host-side rope tables (feature-major, including 1/WSCALE descale)."""
    inv_freq = 1.0 / (10000.0 ** (np.arange(0, D, 2, dtype=np.float32) / D))
    cos_all, sin_all = [], []
    for s in range(NCORES):
        pos = np.arange(TK, dtype=np.float32) + (TS * s - W)
        pos = np.maximum(pos, 0.0)  # halo before seq start: masked later
        f0 = pos[None, :] * inv_freq[:, None]          # (32, TK)
        c32 = np.cos(f0) / WSCALE
        s32 = np.sin(f0) / WSCALE
        cosT = np.concatenate([c32, c32], axis=0)      # (64, TK)
        sinTs = np.concatenate([-s32, s32], axis=0)    # (64, TK) signed
        cos_all.append(np.concatenate([cosT, cosT], axis=0).astype(BF16))
        sin_all.append(np.concatenate([sinTs, sinTs], axis=0).astype(BF16))
    return cos_all, sin_all


PEN_C = 112.0  # per-step unit; /8 sim scale -> -14/step (e4m3 max 240 caps 2C)


def _build_pen():
    """Additive window-mask penalty factors, fp8 DoubleRow layout.

    sim[j, i] += sum_m A[m, j] * B[m, i] = -C*((i-j)+ + (j-i-64)+ [+ pad]),
    so exp(sim/8) <= e^-14 ~ 8e-7 outside the causal window. A is 0/1,
    B in {0, -C, -2C}; all exactly representable in e4m3 (C=112, 2C=224<240).
    """
    m1 = np.arange(64)[:, None]
    j = np.arange(128)[None, :]
    i = np.arange(64)[None, :]
    A = np.zeros((128, 128), np.float32)
    A[0:64] = j <= m1
    A[64:128] = j >= m1 + 65
    Brest = np.zeros((128, 64), np.float32)
    Brest[0:64] = -PEN_C * (i > m1)
    Brest[64:128] = -PEN_C * (i <= m1)
    Bfirst = Brest.copy()
    Bfirst[0:64] -= PEN_C  # core 0 chunk 0: mask the zero-padded halo keys
    penA_np = A.astype(BF16)
    penB0 = np.concatenate([Brest, Bfirst], axis=1).astype(BF16)
    penBr = np.concatenate([Brest, Brest], axis=1).astype(BF16)
    return penA_np, penB0, penBr


def _split8(w):
    """Split a float array into fp8e4 hi + lo (residual), returns fp8 pair."""
    hi = w.astype(E4)
    lo = (w - hi.astype(np.float32)).astype(E4)
    return hi, lo


def _build_bass():
    nc = bacc.Bacc()
    xp = nc.dram_tensor("xp", [B, TK, DIM], BF, kind="ExternalInput")
    w8h = nc.dram_tensor("w8h", [128, 8, 3 * DIM], FP8, kind="ExternalInput")
    w8l = nc.dram_tensor("w8l", [128, 8, DIM], FP8, kind="ExternalInput")
    wo8h = nc.dram_tensor("wo8h", [128, 8, DIM], FP8, kind="ExternalInput")
    wo8l = nc.dram_tensor("wo8l", [128, 8, DIM], FP8, kind="ExternalInput")
    cosT = nc.dram_tensor("cosT", [128, TK], BF, kind="ExternalInput")
    sinT = nc.dram_tensor("sinT", [128, TK], BF, kind="ExternalInput")
    penA = nc.dram_tensor("penA", [128, 128], BF, kind="ExternalInput")
    penB = nc.dram_tensor("penB", [128, 128], BF, kind="ExternalInput")
    ident = nc.dram_tensor("ident", [128, 128], BF, kind="ExternalInput")
    perm = nc.dram_tensor("perm", [128, 128], BF, kind="ExternalInput")
    out = nc.dram_tensor("out", [B, TS, DIM], F32, kind="ExternalOutput")

    with tile.TileContext(nc) as tc, ExitStack() as ctx:
        consts = ctx.enter_context(tc.tile_pool(name="consts", bufs=1))
        xpool = ctx.enter_context(tc.tile_pool(name="xpool", bufs=2))
        spool = ctx.enter_context(tc.tile_pool(name="spool", bufs=3))
        xnb_p = ctx.enter_context(tc.tile_pool(name="xnbp", bufs=4))
        x8_p = ctx.enter_context(tc.tile_pool(name="x8p", bufs=2))
        qk_p = ctx.enter_context(tc.tile_pool(name="qkp", bufs=4))
        v_p = ctx.enter_context(tc.tile_pool(name="vp", bufs=1))
        rope_p = ctx.enter_context(tc.tile_pool(name="ropep", bufs=4))
        pT_p = ctx.enter_context(tc.tile_pool(name="pTp", bufs=13))
        rz_p = ctx.enter_context(tc.tile_pool(name="rzp", bufs=5))
        ao_p = ctx.enter_context(tc.tile_pool(name="aop", bufs=1))
        osb_p = ctx.enter_context(tc.tile_pool(name="osbp", bufs=4))
        psP = ctx.enter_context(tc.tile_pool(name="psP", bufs=3, space="PSUM"))
        psT = ctx.enter_context(tc.tile_pool(name="psT", bufs=1, space="PSUM"))
        psW = ctx.enter_context(tc.tile_pool(name="psW", bufs=2, space="PSUM"))
        psS = ctx.enter_context(tc.tile_pool(name="psS", bufs=1, space="PSUM"))
        psZ = ctx.enter_context(tc.tile_pool(name="psZ", bufs=1, space="PSUM"))

        # --- constants: ident first (needed by first transpose), then fp8
        # weights on the scalar queue, small tables on sync, wo last ---
        id_sb = consts.tile([128, 128], BF)
        nc.sync.dma_start(out=id_sb, in_=ident[:, :])
        w8h_sb = consts.tile([128, 8, 3 * DIM], FP8)
        w8l_sb = consts.tile([128, 8, DIM], FP8)
        cos_sb = consts.tile([128, TK], BF)
        sin_sb = consts.tile([128, TK], BF)
        penA_sb = consts.tile([128, 128], BF)
        penB_sb = consts.tile([128, 128], BF)
        perm_sb = consts.tile([128, 128], BF)
        # The cost model serializes ALL DMA transfers on one engine, so the
        # big weight/table loads must NOT get in front of batch-0's x tiles.
        # Everything rides the sync queue (FIFO): per phase-A group of batch
        # 0 we interleave the next weight chunks after that group's x DMAs.
        _const_dmas = [
            lambda: nc.sync.dma_start(out=w8h_sb[:, 0:2, :], in_=w8h[:, 0:2, :]),
            lambda: nc.sync.dma_start(out=w8l_sb[:, 0:2, :], in_=w8l[:, 0:2, :]),
            lambda: nc.sync.dma_start(out=w8h_sb[:, 2:4, :], in_=w8h[:, 2:4, :]),
            lambda: nc.sync.dma_start(out=w8l_sb[:, 2:4, :], in_=w8l[:, 2:4, :]),
            lambda: nc.sync.dma_start(out=w8h_sb[:, 4:6, :], in_=w8h[:, 4:6, :]),
            lambda: nc.sync.dma_start(out=w8l_sb[:, 4:6, :], in_=w8l[:, 4:6, :]),
            lambda: nc.sync.dma_start(out=w8h_sb[:, 6:8, :], in_=w8h[:, 6:8, :]),
            lambda: nc.sync.dma_start(out=w8l_sb[:, 6:8, :], in_=w8l[:, 6:8, :]),
            lambda: nc.sync.dma_start(out=cos_sb, in_=cosT[:, :]),
            lambda: nc.sync.dma_start(out=sin_sb, in_=sinT[:, :]),
            lambda: nc.sync.dma_start(out=penA_sb, in_=penA[:, :]),
            lambda: nc.sync.dma_start(out=penB_sb, in_=penB[:, :]),
            lambda: nc.sync.dma_start(out=perm_sb, in_=perm[:, :]),
        ]
        ones_sb = consts.tile([128, W], BF)
        nc.vector.memset(ones_sb, WSCALE / 8.0)  # -> norm output = 8*ao (fp8-friendly)
        magic_sb = consts.tile([128, 4], mybir.dt.uint32)
        nc.vector.memset(magic_sb, 0x5F3759DF)  # rsqrt bit-hack seed
        one_u32 = consts.tile([128, 4], mybir.dt.uint32)
        nc.vector.memset(one_u32, 1)
        wo8h_sb = consts.tile([128, 8, DIM], FP8)
        wo8l_sb = consts.tile([128, 8, DIM], FP8)
        _const_dmas.append(lambda: nc.sync.dma_start(out=wo8h_sb, in_=wo8h[:, :, :]))
        _const_dmas.append(lambda: nc.sync.dma_start(out=wo8l_sb, in_=wo8l[:, :, :]))

        ntt = (TK + 127) // 128  # 9 token tiles (last has 64 rows)

        for b in range(B):
            # ---- Phase A: load x, rmsnorm, transpose, split to fp8 hi/lo ----
            x8h = x8_p.tile([128, 8, TK], FP8, tag="x8h", name="x8h", bufs=1)
            x8l = x8_p.tile([128, 8, TK], FP8, tag="x8l", name="x8l", bufs=1)
            # rmsnorm in groups of 3 token tiles: stats per tile, then ONE
            # batched rsqrt (bit-hack + 2 Newton steps, DVE-only: no Act Sqrt
            # -> Act needs only {exp, copy} -> a single act-table set, one
            # LoadActFuncSet instead of per-batch Sqrt<->Exp table thrash).
            for g0 in range(0, ntt, 3):
                gtt = list(range(g0, min(g0 + 3, ntt)))
                gn = len(gtt)
                x_ts = {}
                ms = spool.tile([128, 4], F32, tag="ms")
                yv = spool.tile([128, 4], F32, tag="yv")
                tv = spool.tile([128, 4], F32, tag="tv")
                # last tile has pt=64: init so Newton never reads uninit rows
                nc.vector.memset(ms[:, :gn], 1.0)
                for gi, tt in enumerate(gtt):
                    pt = min(128, TK - tt * 128)
                    x_t = xpool.tile([128, DIM], BF, tag=f"x_t{tt % 4}")
                    x_ts[tt] = x_t
                    nc.sync.dma_start(
                        out=x_t[:pt], in_=xp[b, tt * 128 : tt * 128 + pt, :]
                    )
                    stats = spool.tile([128, 2, 6], F32, tag="stats")
                    mv = spool.tile([128, 2], F32, tag="mv")
                    for g in range(2):
                        nc.vector.bn_stats(
                            out=stats[:pt, g], in_=x_t[:pt, g * 512 : (g + 1) * 512]
                        )
                    nc.vector.bn_aggr(out=mv[:pt], in_=stats[:pt])
                    # mean(x^2) = var + mean^2
                    nc.vector.tensor_tensor(
                        out=ms[:pt, gi : gi + 1], in0=mv[:pt, 0:1], in1=mv[:pt, 0:1],
                        op=mybir.AluOpType.mult,
                    )
                    nc.vector.tensor_add(
                        ms[:pt, gi : gi + 1], ms[:pt, gi : gi + 1], mv[:pt, 1:2]
                    )
                if b == 0:
                    gidx = g0 // 3
                    lo, hi = gidx * 4, (gidx * 4 + 4 if gidx < 2 else len(_const_dmas))
                    for fn in _const_dmas[lo:hi]:
                        fn()
                nc.vector.tensor_scalar_add(ms[:, :gn], ms[:, :gn], EPS)
                msu = ms.bitcast(mybir.dt.uint32)
                yu = yv.bitcast(mybir.dt.uint32)
                nc.vector.tensor_tensor(
                    out=yu[:, :gn], in0=msu[:, :gn], in1=one_u32[:, :gn],
                    op=mybir.AluOpType.logical_shift_right,
                )
                nc.vector.tensor_tensor(
                    out=yu[:, :gn], in0=magic_sb[:, :gn], in1=yu[:, :gn],
                    op=mybir.AluOpType.subtract,
                )
                for _ in range(2):
                    nc.vector.tensor_tensor(
                        out=tv[:, :gn], in0=yv[:, :gn], in1=yv[:, :gn],
                        op=mybir.AluOpType.mult,
                    )
                    nc.vector.tensor_tensor(
                        out=tv[:, :gn], in0=tv[:, :gn], in1=ms[:, :gn],
                        op=mybir.AluOpType.mult,
                    )
                    nc.vector.tensor_scalar(
                        out=tv[:, :gn], in0=tv[:, :gn], scalar1=-0.5, scalar2=1.5,
                        op0=mybir.AluOpType.mult, op1=mybir.AluOpType.add,
                    )
                    nc.vector.tensor_tensor(
                        out=yv[:, :gn], in0=yv[:, :gn], in1=tv[:, :gn],
                        op=mybir.AluOpType.mult,
                    )
                for gi, tt in enumerate(gtt):
                    pt = min(128, TK - tt * 128)
                    x_t = x_ts[tt]
                    xnb = xnb_p.tile([128, DIM], BF, tag="xnb")
                    nc.vector.tensor_scalar_mul(
                        xnb[:pt], in0=x_t[:pt], scalar1=yv[:pt, gi : gi + 1]
                    )
                    tps = psT.tile([128, 1024], BF, tag="tps", name="tps")
                    for kf in range(8):
                        nc.tensor.transpose(
                            tps[:, kf * 128 : kf * 128 + pt],
                            xnb[:pt, kf * 128 : (kf + 1) * 128],
                            id_sb[:pt, :pt],
                        )
                    # hi: Act cast-copy psum->fp8 (strided dest over 8 kf slabs)
                    src = tps.rearrange("p (k8 t) -> p k8 t", k8=8)[:, :, :pt]
                    dsth = x8h[:, :, tt * 128 : tt * 128 + pt]
                    nc.scalar.copy(out=dsth, in_=src)
                    # lo: DVE subtract -> fp8
                    dstl = x8l[:, :, tt * 128 : tt * 128 + pt]
                    nc.vector.tensor_tensor(
                        out=dstl, in0=src, in1=dsth, op=mybir.AluOpType.subtract
                    )

            # ---- Phase B: v projection (fp8 DoubleRow, token-major) ----
            v_sb = [v_p.tile([128, DIM], BF, tag=f"v{t}", name=f"v{t}") for t in range(ntt)]
            v2_sb = [v_p.tile([128, DIM], BF, tag=f"w{t}", name=f"w{t}") for t in range(8)]
            for tt in range(ntt):
                pt = min(128, TK - tt * 128)
                for nch in range(2):
                    ps = psP.tile([128, 512], F32, tag="pj", name="psv")
                    # j-major so the first terms only need the first weight
                    # chunks (weight DMAs stream in behind batch-0 phase A)
                    terms = [(j, lt, rt, rc)
                             for j in range(4)
                             for lt, rt, rc in ((x8h, w8h_sb, 2 * DIM),
                                                (x8l, w8h_sb, 2 * DIM),
                                                (x8h, w8l_sb, 0))]
                    for i, (j, lt, rt, rc) in enumerate(terms):
                        nc.tensor.matmul(
                            ps[:pt],
                            lhsT=lt[:, 2 * j : 2 * j + 2, tt * 128 : tt * 128 + pt],
                            rhs=rt[:, 2 * j : 2 * j + 2,
                                   rc + nch * 512 : rc + (nch + 1) * 512],
                            start=i == 0,
                            stop=i == len(terms) - 1,
                            perf_mode=mybir.MatmulPerfMode.DoubleRow,
                        )
                    nc.scalar.copy(
                        out=v_sb[tt][:pt, nch * 512 : (nch + 1) * 512], in_=ps[:pt]
                    )
            # shifted copy for odd window chunks (single aligned K=128 matmuls)
            for m2 in range(8):
                nc.gpsimd.dma_start(out=v2_sb[m2][0:64, :], in_=v_sb[m2][64:128, :])
                nc.gpsimd.dma_start(out=v2_sb[m2][64:128, :], in_=v_sb[m2 + 1][0:64, :])

            # ---- Phase C: per head-pair: q/k projection + rope + attention ----
            ao8h = ao_p.tile([128, 8, TS], FP8, tag="ao8h", name="ao8h")
            ao8l = ao_p.tile([128, 8, TS], FP8, tag="ao8l", name="ao8l")
            for ht in range(8):
                qTp = qk_p.tile([128, TS], BF, tag="qTp", name="qTp")
                kTp = qk_p.tile([128, TK], BF, tag="kTp", name="kTp")
                for isq in (False, True):
                    ft = ht if isq else 8 + ht
                    tok0 = W if isq else 0
                    toklen = TS if isq else TK
                    dst = qTp if isq else kTp
                    for c0 in range(0, toklen, 512):
                        cl = min(512, toklen - c0)
                        ps = psP.tile([128, 512], F32, tag="pj", name="psqk")
                        # 1-term fp8: activation quantization error on q/k is
                        # smoothed by softmax; measured end-to-end rel err
                        # 0.0122 (gate 2e-2) vs 0.0083 for the 2-term version,
                        # at HALF the PE cost of the q/k projections.
                        first, last = (0, 0), (3, 0)
                        for j in range(4):
                            for ti, lt in enumerate((x8h,)):
                                nc.tensor.matmul(
                                    ps[:, :cl],
                                    lhsT=w8h_sb[:, 2 * j : 2 * j + 2,
                                                ft * 128 : (ft + 1) * 128],
                                    rhs=lt[:, 2 * j : 2 * j + 2,
                                           tok0 + c0 : tok0 + c0 + cl],
                                    start=(j, ti) == first,
                                    stop=(j, ti) == last,
                                    perf_mode=mybir.MatmulPerfMode.DoubleRow,
                                )
                        # rope: dst = qc*cos + (perm@qc)*sin
                        ca = tok0 + c0
                        qc = rope_p.tile([128, 512], BF, tag="ropeqc", name="qc", bufs=6)
                        nc.scalar.copy(out=qc[:, :cl], in_=ps[:, :cl])
                        qsw = psW.tile([128, 512], F32, tag="qsw", name="qsw")
                        nc.tensor.matmul(qsw[:, :cl], lhsT=perm_sb, rhs=qc[:, :cl],
                                         start=True, stop=True)
                        t1 = rope_p.tile([128, 512], BF, tag="ropet1", name="t1")
                        t1e = nc.gpsimd if (c0 // 512 + ft) % 2 == 0 else nc.vector
                        t1e.tensor_mul(t1[:, :cl], qc[:, :cl],
                                       cos_sb[:, ca : ca + cl])
                        t2 = rope_p.tile([128, 512], BF, tag="ropet2", name="t2")
                        nc.vector.tensor_mul(t2[:, :cl], qsw[:, :cl],
                                             sin_sb[:, ca : ca + cl])
                        adde = nc.vector if (c0 // 512 + ft) % 2 == 0 else nc.gpsimd
                        adde.tensor_add(dst[:, c0 : c0 + cl],
                                        t1[:, :cl], t2[:, :cl])

                for cb in range(2):
                    # both heads of the pair share zb/pv tiles: head A rows
                    # 0:64, head B rows 64:128 -> one recip + one dmul per cb
                    pTs = []
                    for h in (2 * ht, 2 * ht + 1):
                        hr = (h % 2) * 64
                        sim = psS.tile([128, 512], F32, tag="sim", name="sim")
                        for cc in range(8):
                            gc = cb * 8 + cc
                            nc.tensor.matmul(
                                sim[:, cc * 64 : (cc + 1) * 64],
                                lhsT=kTp[hr : hr + 64, gc * 64 : gc * 64 + 128],
                                rhs=qTp[hr : hr + 64, gc * 64 : (gc + 1) * 64],
                                start=True, stop=False,
                                tile_position=(hr, 0),
                            )
                            # window mask as an additive staircase penalty:
                            # sim[j,i] += -C*((i-j)+ + (j-i-64)+ [+ pad]),
                            # rank-128 product of 0/1 matrices (bf16: fp8-DR in a
                            # bf16 tile_position group faults real HW at
                            # scale) -> kills the post-exp mask multiply and
                            # one chain hop in the softmax chain.
                            poff = 64 if gc == 0 else 0
                            nc.tensor.matmul(
                                sim[:, cc * 64 : (cc + 1) * 64],
                                lhsT=penA_sb[:, :],
                                rhs=penB_sb[:, poff : poff + 64],
                                start=False, stop=True,
                            )
                        pT = pT_p.tile([128, 512], BF, tag="pT")
                        nc.scalar.activation(
                            out=pT[:, :], in_=sim[:, :],
                            func=mybir.ActivationFunctionType.Exp, scale=1.0 / 8.0,
                        )
                        pTs.append(pT)
                    zb = psZ.tile([128, 512], F32, tag="zv", name="zb")
                    pv = psZ.tile([128, 512], F32, tag="zv", name="pv")
                    for hi, h in enumerate((2 * ht, 2 * ht + 1)):
                        hr = hi * 64
                        nc.tensor.matmul(
                            zb[hr : hr + 64, :], lhsT=ones_sb[:, :], rhs=pTs[hi][:, :],
                            start=True, stop=True, tile_position=(0, hr),
                        )
                        for cc in range(8):
                            gc = cb * 8 + cc
                            hc = slice(h * 64, (h + 1) * 64)
                            vt = (v_sb[gc // 2][:, hc] if gc % 2 == 0
                                  else v2_sb[(gc - 1) // 2][:, hc])
                            nc.tensor.matmul(
                                pv[hr : hr + 64, cc * 64 : (cc + 1) * 64],
                                lhsT=vt, rhs=pTs[hi][:, cc * 64 : (cc + 1) * 64],
                                start=True, stop=True, tile_position=(0, hr),
                            )
                    rz = rz_p.tile([128, 512], BF, tag="rz", name="rz")
                    with nc.allow_low_precision(reason="1/Z in bf16; rel err budget 2e-2"):
                        nc.vector.reciprocal(rz[:, :], zb[:, :])
                    # normalize -> 8*ao (ones=8 descale), split fp8 hi/lo for
                    # the DoubleRow out-projection (3-term, rel-err neutral)
                    tmp = rz_p.tile([128, 512], BF, tag="aotmp", name="aotmp")
                    nc.vector.tensor_mul(tmp[:, :], pv[:, :], rz[:, :])
                    hslc = (slice(None), ht, slice(cb * 512, (cb + 1) * 512))
                    nc.scalar.copy(out=ao8h[hslc], in_=tmp[:, :])
                    nc.gpsimd.tensor_tensor(
                        out=ao8l[hslc], in0=tmp[:, :], in1=ao8h[hslc],
                        op=mybir.AluOpType.subtract,
                    )

            # ---- Phase D: output projection (fp8 DoubleRow 3-term) ----
            for tt in range(8):
                for nch in range(2):
                    ps = psP.tile([128, 512], F32, tag="pj", name="pso")
                    oterms = ([(j, ao8h, rt) for j in range(4)
                               for rt in (wo8h_sb, wo8l_sb)]
                              + [(j, ao8l, wo8h_sb) for j in range(4)])
                    for i, (j, lt, rt) in enumerate(oterms):
                        nc.tensor.matmul(
                            ps[:, :],
                            lhsT=lt[:, 2 * j : 2 * j + 2, tt * 128 : (tt + 1) * 128],
                            rhs=rt[:, 2 * j : 2 * j + 2, nch * 512 : (nch + 1) * 512],
                            start=i == 0,
                            stop=i == len(oterms) - 1,
                            perf_mode=mybir.MatmulPerfMode.DoubleRow,
                        )
                    osb = osb_p.tile([128, 512], F32, tag="osb", name="osb")
                    # descale the 8x (ao) * 64x (wo) pre-scales
                    nc.scalar.activation(
                        out=osb[:, :], in_=ps[:, :],
                        func=mybir.ActivationFunctionType.Copy,
                        scale=1.0 / (8.0 * WSCALE),
                    )
                    nc.scalar.dma_start(
                        out=out[b, tt * 128 : (tt + 1) * 128,
                                nch * 512 : (nch + 1) * 512],
                        in_=osb[:, :],
                    )
    nc.finalize()
    return nc


_NC_CACHE = None


def _host_inputs(x, w_qkv, w_o):
    xpad = np.concatenate([np.zeros((B, W, DIM), np.float32), x], axis=1)
    ws = np.asarray(w_qkv, np.float32) * WSCALE
    # DoubleRow layout: w8[p, 2j+kt, m] = ws[(2j+kt)*128 + p, m]
    w3 = ws.reshape(8, 128, 3 * DIM)          # (kf, p, m)
    w3 = np.ascontiguousarray(w3.transpose(1, 0, 2))  # (p, kf, m)
    w8h, w8l_full = _split8(w3)
    w8l = np.ascontiguousarray(w8l_full[:, :, 2 * DIM :])
    wos = np.asarray(w_o, np.float32) * WSCALE
    wos = wos.reshape(8, 128, DIM)
    wos = np.ascontiguousarray(wos.transpose(1, 0, 2))  # (p, kt, m)
    wo8h_np, wo8l_np = _split8(wos)
    ident = np.eye(128, dtype=BF16)
    perm_np = np.zeros((128, 128), dtype=BF16)
    for r in range(128):
        perm_np[r ^ 32, r] = 1
    cos_all, sin_all = _build_tables()
    penA_np, penB0, penBr = _build_pen()
    in_maps = []
    for s in range(NCORES):
        in_maps.append(
            {
                "xp": np.ascontiguousarray(xpad[:, TS * s : TS * s + TK, :]).astype(BF16),
                "w8h": w8h,
                "w8l": w8l,
                "wo8h": wo8h_np,
                "wo8l": wo8l_np,
                "cosT": cos_all[s],
                "sinT": sin_all[s],
                "penA": penA_np,
                "penB": penB0 if s == 0 else penBr,
                "ident": ident,
                "perm": perm_np,
            }
        )
    return in_maps


def kernel(x, w_norm, w_qkv, w_o, heads, window_size):
    global _NC_CACHE
    assert int(heads) == HEADS and int(window_size) == W
    x = np.asarray(x, np.float32)
    assert x.shape == (B, N, DIM)
    # note: w_norm is all-ones per the problem spec; rmsnorm weight folded out.
    in_maps = _host_inputs(x, w_qkv, w_o)
    if _NC_CACHE is None:
        _NC_CACHE = _build_bass()
    res = run_bass_kernel_spmd(_NC_CACHE, in_maps, list(range(NCORES)))
    outs = [np.asarray(r["out"], np.float32) for r in res.results]
    return np.concatenate(outs, axis=1)


if __name__ == "__main__":
    pass

